# revision 26
# baseline (speedup 1.0000x reference)
"""HashEmbedding (hash -> gather -> sum-pool) on 8 TRN2 NeuronCores.

Strategy: batch-data-parallel (each core owns 512 of the 4096 batch rows
and a full copy of the table in its local HBM). Per-core gather traffic
is 512*200 = 102,400 rows; no collectives.

The gather primitive is the ANT `dma_gather` (SWDGE CounterMachine).
Profiling the single-queue f32 baseline showed GpSimd (the Q7 pair doing
descriptor generation) 97% busy at 1.18 ms while DMA/PE idled, so this
version attacks descriptor generation and the fp32 matmul rate:

 1. 4 SWDGE queues: the Q7 ucode serves queue q with core pair (2q,
    2q+1), so round-robining the (window, batch-group) gather calls over
    queue_num 0-3 runs descriptor generation on 4 core pairs
    concurrently instead of 1.
 2. bf16 table: the host converts the f32 table to bf16 once. Gathered
    rows shrink 512B -> 256B (half the HBM traffic) and the pooling
    matmuls run at full bf16 PE rate instead of 1/4 fp32 rate.
 3. Exact-count gathers: window-bucket capacity padding (CAP=1024 vs
    mean 839) is marked with trailing -1 indices, which the Q7 ucode
    trims before generating descriptors; the per-call valid count is
    read at runtime from a counts tensor into a Pool register
    (num_idxs_reg), as the sequencer-side ring accounting requires.
    This skips ~18% of descriptors and gather bytes.

Pooling is unchanged: per gathered chunk of 128 rows, a 0/1 assignment
matrix A[p, m] = (slot[p] == m) is built on the DVE (bf16), and
psum[m, d] += A^T @ G accumulates the sum-pool across windows. Padding
slots are -1 so they match no column. Gather tiles are memset once per
pool buffer (first two windows) so skipped tail positions never hold
NaN bit patterns (0 * NaN would poison the psum).
"""

import sys

if "/opt/trn_rl_repo" not in sys.path:
    sys.path.insert(0, "/opt/trn_rl_repo")

import numpy as np

B, H, D, V = 4096, 200, 128, 1_000_000
NCORES = 8
BPC = B // NCORES              # 512 batch rows per core
NPASS = 4                      # batch groups of 128 rows (PSUM M limit)
WBITS = 15
W = 1 << WBITS                 # 32768-row window (int16 index limit)
NW = (V + W - 1) // W          # 31 windows
CAP = 1024                     # capacity per (window, pass); mu=839, sigma=28
CALL_IDX = NPASS * CAP         # 4096 indices per window
CHUNKS = CALL_IDX // 128       # 32 matmul chunks per window
NQ = 4                         # SWDGE queues (ucode MAX_SWDGE_QUEUES)
# Each dma_gather call covers TWO batch groups (2048 indices: group 2h
# padded to 1024 with zeros, group 2h+1 padded with trimmed -1s), so a
# window is 2 calls. Consecutive windows use disjoint queue pairs
# ({0,1} vs {2,3}), so each Q7 core pair generates descriptors for one
# call every other window -- per-call fixed costs (instruction load,
# ring resets, respond) amortize over 2x the indices.
NCALL = 2                      # calls per window
CIDX = 2 * CAP                 # 2048 indices per call
CSUB = CIDX // 128             # 16 matmul chunks per call
GBUFS = 8                      # gather-tile pool depth (4 windows in flight)
# First GBUFS/NCALL windows run full-CAP (padding gathers row 0 of the
# window instead of being trimmed) so every gather-pool buffer is fully
# overwritten on first touch -- masked-out tail positions afterwards
# always hold finite (stale) data, never SBUF boot garbage (0 * NaN
# would poison the psum accumulation).
FULL_WINDOWS = GBUFS // NCALL

_cache: dict = {}


def _f32_to_bf16(a):
    """Round-to-nearest-even f32 -> bf16, as uint16 bits (pure numpy)."""
    b = np.ascontiguousarray(a, dtype=np.float32).view(np.uint32)
    rounding = np.uint32(0x7FFF) + ((b >> np.uint32(16)) & np.uint32(1))
    return ((b + rounding) >> np.uint32(16)).astype(np.uint16)


def _host_prep(x_core):
    """Hash + window-sort one core's ids.

    Returns (loc16 [NW,128,256] wrapped (zero padding in the interior
             group of each call, -1 trimmed padding at the tail),
             slotf [NW,128,CHUNKS] f32 with -1 padding,
             counts [1, NW*NCALL] int32 per-call valid counts)."""
    idx = (
        (x_core.astype(np.uint32).ravel() * np.uint32(2654435761))
        % np.uint32(V)
    ).astype(np.int32)                       # [BPC*H]
    b = np.repeat(np.arange(BPC, dtype=np.int32), H)
    win = idx >> WBITS
    loc = idx & (W - 1)
    grp = b >> 7                              # pass
    slot = b & 127

    bucket = win * NPASS + grp
    order = np.argsort(bucket, kind="stable")
    bs, ls, ss = bucket[order], loc[order], slot[order]
    counts = np.bincount(bucket, minlength=NW * NPASS)
    if counts.max() > CAP:
        raise RuntimeError(f"window bucket overflow: {counts.max()} > {CAP}")
    if counts.min() < 1:
        raise RuntimeError("empty (window, pass) bucket")
    starts = np.zeros(NW * NPASS, dtype=np.int64)
    starts[1:] = np.cumsum(counts)[:-1]
    rank = np.arange(bs.size) - starts[bs]

    loc_arr = np.full((NW, NPASS, CAP), -1, dtype=np.int16)
    slot_arr = np.full((NW, NPASS, CAP), -1.0, dtype=np.float32)
    loc_arr[bs // NPASS, bs % NPASS, rank] = ls.astype(np.int16)
    slot_arr[bs // NPASS, bs % NPASS, rank] = ss.astype(np.float32)

    cn = counts.astype(np.int32).reshape(NW, NPASS)
    # call h = [grp 2h (zero interior padding, never trimmed),
    #           grp 2h+1 (-1 tail padding, trimmed by the Q7 ucode)]
    loc_arr[:, 0::2][loc_arr[:, 0::2] < 0] = 0
    cnts = CAP + cn[:, 1::2]                          # [NW, NCALL]
    # first-touch windows: full-CAP gathers (pad with window row 0)
    loc_arr[:FULL_WINDOWS][loc_arr[:FULL_WINDOWS] < 0] = 0
    cnts[:FULL_WINDOWS] = CIDX

    flat_loc = loc_arr.reshape(NW, CALL_IDX)
    # SWDGE wrapped layout: within a (window, pass) call, position i sits
    # at [partition i%16, col grp*(CAP//16) + i//16], replicated to all 8
    # Q7-core partition groups (each queue's core pair reads its own).
    wrapped = flat_loc.reshape(NW, NPASS, CAP // 16, 16).transpose(0, 3, 1, 2)
    wrapped = wrapped.reshape(NW, 16, CALL_IDX // 16)
    loc16 = np.tile(wrapped, (1, 8, 1)).copy()            # [NW, 128, 256]
    # slot layout matching gather output: position i -> (p=i%128, c=i//128)
    slotf = (
        slot_arr.reshape(NW, CHUNKS, 128).transpose(0, 2, 1).copy()
    )                                                      # [NW, 128, CHUNKS]
    return loc16, slotf, cnts.reshape(1, NW * NCALL)


def _build():
    import concourse.tile as tile
    from concourse import bacc, mybir

    i16, i32, f32 = mybir.dt.int16, mybir.dt.int32, mybir.dt.float32
    bf16 = mybir.dt.bfloat16
    Alu = mybir.AluOpType

    nc = bacc.Bacc(
        "TRN2",
        target_bir_lowering=False,
        debug=False,
        enable_asserts=False,
        # SWDGE descriptor carveout: a dma_gather call of N descriptors
        # needs >= 32*N bytes here (HW-verified on the f32 baseline).
        # Queues write their rings to different partition groups, so the
        # same carveout serves all 4 queues. 2048-index calls also need
        # single_packet=False (64-descriptor packet ceiling per engine).
        dynamic_dma_scratch_size=65536,
        num_swdge_queues=NQ,
    )
    tb_ap = nc.dram_tensor("table", [NW * W, D], bf16, kind="ExternalInput").ap()
    loc_ap = nc.dram_tensor(
        "loc16", [NW, 128, CALL_IDX // 16], i16, kind="ExternalInput"
    ).ap()
    slot_ap = nc.dram_tensor(
        "slotf", [NW, 128, CHUNKS], f32, kind="ExternalInput"
    ).ap()
    cnt_ap = nc.dram_tensor(
        "counts", [1, NW * NCALL], i32, kind="ExternalInput"
    ).ap()
    out_ap = nc.dram_tensor("out", [BPC, D], f32, kind="ExternalOutput").ap()

    with tile.TileContext(nc) as tc:
        with (
            tc.tile_pool(name="iop", bufs=1) as iop,
            tc.tile_pool(name="inp", bufs=8) as inp,
            tc.tile_pool(name="gp", bufs=GBUFS) as gp,
            tc.tile_pool(name="ap_", bufs=5) as ap_,
            tc.tile_pool(name="op", bufs=2) as op,
            tc.tile_pool(name="pp", bufs=1, space="PSUM") as pp,
        ):
            iota_i = iop.tile([128, 128], i32, name="iota_i")
            nc.gpsimd.iota(iota_i[:], [[1, 128]], base=0, channel_multiplier=0)
            # iota lives in PSUM: the per-window A-build then has only ONE
            # SBUF input stream, so the DVE stays in 1-port mode and never
            # takes the SBUF port pair it shares with GpSimd. In 2-port mode
            # each 4.4us A-build fully blocked Q7 descriptor generation
            # (exclusive lock), stretching the window cadence.
            iota_p = pp.tile([128, 128], f32, name="iota_p", tag="iota_p")
            nc.vector.tensor_copy(iota_p[:], iota_i[:])

            ct = iop.tile([1, NW * NCALL], i32, name="ct")
            nc.sync.dma_start(out=ct[:], in_=cnt_ap[:, :])
            cregs = [
                nc.gpsimd.alloc_register(f"gather_cnt{q}") for q in range(NCALL)
            ]

            psums = [
                pp.tile([128, D], f32, name=f"ps{g}", tag=f"ps{g}")
                for g in range(NPASS)
            ]

            for w in range(NW):
                lt = inp.tile([128, CALL_IDX // 16], i16, name="lt", tag="lt")
                nc.sync.dma_start(out=lt[:], in_=loc_ap[w])
                st = inp.tile([128, CHUNKS], f32, name="st", tag="st")
                nc.sync.dma_start(out=st[:], in_=slot_ap[w])

                A = ap_.tile([128, CHUNKS, 128], bf16, name="A", tag="A")
                iota_bc = iota_p[:].unsqueeze(1).broadcast_to([128, CHUNKS, 128])
                st_bc = st[:].unsqueeze(2).broadcast_to([128, CHUNKS, 128])
                nc.vector.tensor_tensor(A[:], iota_bc, st_bc, Alu.is_equal)

                # one TENSOR_LOAD fills both count registers for the window
                nc.gpsimd.reg_load(
                    cregs, ct[0:1, w * NCALL : (w + 1) * NCALL]
                )
                for h in range(NCALL):
                    # call h covers batch groups {2h, 2h+1}; consecutive
                    # windows alternate queue pairs so all 4 Q7 core pairs
                    # generate descriptors concurrently across windows
                    g = gp.tile([128, CSUB, D], bf16, name="g", tag="g")
                    nc.gpsimd.dma_gather(
                        g[:],
                        tb_ap[w * W : (w + 1) * W, :],
                        lt[:, h * (CIDX // 16) : (h + 1) * (CIDX // 16)],
                        CIDX,
                        cregs[h],
                        D,
                        queue_num=2 * (w % 2) + h,
                        single_packet=False,
                    )
                    for c in range(CSUB):
                        grp = 2 * h + (c >= CSUB // 2)
                        nc.tensor.matmul(
                            psums[grp][:],
                            A[:, h * CSUB + c, :],
                            g[:, c, :],
                            start=(w == 0 and c % (CSUB // 2) == 0),
                            stop=(w == NW - 1 and c % (CSUB // 2) == CSUB // 2 - 1),
                        )

            for grp in range(NPASS):
                outs = op.tile([128, D], f32, name="outs", tag="outs")
                nc.vector.tensor_copy(outs[:], psums[grp][:])
                nc.sync.dma_start(
                    out=out_ap[grp * 128 : (grp + 1) * 128, :], in_=outs[:]
                )

    nc.compile()
    return nc


def _run(x, table, trace=False):
    from concourse.bass_utils import run_bass_kernel_spmd

    if "nc" not in _cache:
        _cache["nc"] = _build()
    nc = _cache["nc"]

    x_np = np.asarray(x)
    # pad the table to NW*W rows so every gather window is a full 32768,
    # converted to bf16 (rel tolerance is 2e-2; bf16 error ~4e-3)
    import ml_dtypes

    tb = np.zeros((NW * W, D), dtype=np.uint16)
    tb[:V] = _f32_to_bf16(np.asarray(table, dtype=np.float32))
    tb16 = tb.view(ml_dtypes.bfloat16)
    in_maps = []
    for c in range(NCORES):
        loc16, slotf, cnts = _host_prep(x_np[c * BPC : (c + 1) * BPC])
        in_maps.append(
            {"table": tb16, "loc16": loc16, "slotf": slotf, "counts": cnts}
        )
    res = run_bass_kernel_spmd(nc, in_maps, list(range(NCORES)), trace=trace)
    out = np.concatenate(
        [res.results[c]["out"] for c in range(NCORES)], axis=0
    ).astype(np.float32)
    return out, res


def kernel(x, table):
    out, _ = _run(x, table, trace=False)
    return out


# revision 33
# speedup vs baseline: 1.8323x; 1.8323x over previous
"""HashEmbedding (hash -> gather -> sum-pool) on 8 TRN2 NeuronCores.

Strategy: batch-data-parallel (each core owns 512 of the 4096 batch rows
and a full copy of the table in its local HBM). Per-core gather traffic
is 512*200 = 102,400 rows; no collectives.

The gather primitive is the ANT `dma_gather` (SWDGE CounterMachine).
Profiling the single-queue f32 baseline showed GpSimd (the Q7 pair doing
descriptor generation) 97% busy at 1.18 ms while DMA/PE idled, so this
version attacks descriptor generation and the fp32 matmul rate:

 1. 4 SWDGE queues: the Q7 ucode serves queue q with core pair (2q,
    2q+1), so round-robining the (window, batch-group) gather calls over
    queue_num 0-3 runs descriptor generation on 4 core pairs
    concurrently instead of 1.
 2. bf16 table: the host converts the f32 table to bf16 once. Gathered
    rows shrink 512B -> 256B (half the HBM traffic) and the pooling
    matmuls run at full bf16 PE rate instead of 1/4 fp32 rate.
 3. Exact-count gathers: window-bucket capacity padding (CAP=1024 vs
    mean 839) is marked with trailing -1 indices, which the Q7 ucode
    trims before generating descriptors; the per-call valid count is
    read at runtime from a counts tensor into a Pool register
    (num_idxs_reg), as the sequencer-side ring accounting requires.
    This skips ~18% of descriptors and gather bytes.

Pooling is unchanged: per gathered chunk of 128 rows, a 0/1 assignment
matrix A[p, m] = (slot[p] == m) is built on the DVE (bf16), and
psum[m, d] += A^T @ G accumulates the sum-pool across windows. Padding
slots are -1 so they match no column. Gather tiles are memset once per
pool buffer (first two windows) so skipped tail positions never hold
NaN bit patterns (0 * NaN would poison the psum).
"""

import sys

if "/opt/trn_rl_repo" not in sys.path:
    sys.path.insert(0, "/opt/trn_rl_repo")

import numpy as np

B, H, D, V = 4096, 200, 128, 1_000_000
NCORES = 8
BPC = B // NCORES              # 512 batch rows per core
NPASS = 4                      # batch groups of 128 rows (PSUM M limit)
WBITS = 15
W = 1 << WBITS                 # 32768-row window (int16 index limit)
NW = (V + W - 1) // W          # 31 windows
CAP = 1024                     # capacity per (window, pass); mu=839, sigma=28
CALL_IDX = NPASS * CAP         # 4096 indices per window
CHUNKS = CALL_IDX // 128       # 32 matmul chunks per window
SUBC = CAP // 128              # 8 chunks per (window, pass) call
NQ = 4                         # SWDGE queues (ucode MAX_SWDGE_QUEUES)
GBUFS = 16                     # gather-tile pool depth (4 windows in flight)
# First GBUFS/NPASS windows run full-CAP (padding gathers row 0 of the
# window instead of being trimmed) so every gather-pool buffer is fully
# overwritten on first touch -- masked-out tail positions afterwards
# always hold finite (stale) data, never SBUF boot garbage (0 * NaN
# would poison the psum accumulation).
FULL_WINDOWS = GBUFS // NPASS

_cache: dict = {}


def _f32_to_bf16(a):
    """Round-to-nearest-even f32 -> bf16, as uint16 bits (pure numpy)."""
    b = np.ascontiguousarray(a, dtype=np.float32).view(np.uint32)
    rounding = np.uint32(0x7FFF) + ((b >> np.uint32(16)) & np.uint32(1))
    return ((b + rounding) >> np.uint32(16)).astype(np.uint16)


def _host_prep(x_core):
    """Hash + window-sort one core's ids.

    Returns (loc16 [NW,128,256] wrapped with -1 tail padding,
             sloti [NW,128,CHUNKS] int16 with -1 padding,
             counts [1, NW*NPASS] int32 exact per-call valid counts)."""
    idx = (
        (x_core.astype(np.uint32).ravel() * np.uint32(2654435761))
        % np.uint32(V)
    ).astype(np.int32)                       # [BPC*H]
    b = np.repeat(np.arange(BPC, dtype=np.int32), H)
    win = idx >> WBITS
    loc = idx & (W - 1)
    grp = b >> 7                              # pass
    slot = b & 127

    bucket = win * NPASS + grp
    order = np.argsort(bucket, kind="stable")
    bs, ls, ss = bucket[order], loc[order], slot[order]
    counts = np.bincount(bucket, minlength=NW * NPASS)
    if counts.max() > CAP:
        raise RuntimeError(f"window bucket overflow: {counts.max()} > {CAP}")
    if counts.min() < 1:
        raise RuntimeError("empty (window, pass) bucket")
    starts = np.zeros(NW * NPASS, dtype=np.int64)
    starts[1:] = np.cumsum(counts)[:-1]
    rank = np.arange(bs.size) - starts[bs]

    loc_arr = np.full((NW, NPASS, CAP), -1, dtype=np.int16)
    slot_arr = np.full((NW, NPASS, CAP), -1.0, dtype=np.float32)
    loc_arr[bs // NPASS, bs % NPASS, rank] = ls.astype(np.int16)
    slot_arr[bs // NPASS, bs % NPASS, rank] = ss.astype(np.float32)

    cnts = counts.astype(np.int32).reshape(NW, NPASS)
    # first-touch windows: full-CAP gathers (pad with window row 0)
    loc_arr[:FULL_WINDOWS][loc_arr[:FULL_WINDOWS] < 0] = 0
    cnts[:FULL_WINDOWS] = CAP

    flat_loc = loc_arr.reshape(NW, CALL_IDX)
    # SWDGE wrapped layout: within a (window, pass) call, position i sits
    # at [partition i%16, col grp*(CAP//16) + i//16], replicated to all 8
    # Q7-core partition groups (each queue's core pair reads its own).
    wrapped = flat_loc.reshape(NW, NPASS, CAP // 16, 16).transpose(0, 3, 1, 2)
    wrapped = wrapped.reshape(NW, 16, CALL_IDX // 16)
    loc16 = np.tile(wrapped, (1, 8, 1)).copy()            # [NW, 128, 256]
    # slot layout matching gather output: position i -> (p=i%128, c=i//128)
    sloti = (
        slot_arr.reshape(NW, CHUNKS, 128).transpose(0, 2, 1).copy()
    )                                                      # [NW, 128, CHUNKS]
    return loc16, sloti, cnts.reshape(1, NW * NPASS)


def _build(caps):
    """caps: [NW, NPASS] int per-call static num_idxs (multiple of 16 in
    (896, 1024], so each call still maps to 8 output chunks). Shorter
    static counts shrink the Q7 index-read loop and the trailing -1 trim
    walk, which are the dominant per-call descriptor-generation costs."""
    import concourse.tile as tile
    from concourse import bacc, mybir

    i16, i32, f32 = mybir.dt.int16, mybir.dt.int32, mybir.dt.float32
    bf16 = mybir.dt.bfloat16
    Alu = mybir.AluOpType

    nc = bacc.Bacc(
        "TRN2",
        target_bir_lowering=False,
        debug=False,
        enable_asserts=False,
        # SWDGE descriptor carveout: a dma_gather call of N descriptors
        # needs >= 32*N bytes here (HW-verified on the f32 baseline).
        # Queues write their rings to different partition groups, so the
        # same carveout serves all 4 queues.
        dynamic_dma_scratch_size=32768,
        num_swdge_queues=NQ,
    )
    tb_ap = nc.dram_tensor("table", [NW * W, D], bf16, kind="ExternalInput").ap()
    loc_ap = nc.dram_tensor(
        "loc16", [NW, 128, CALL_IDX // 16], i16, kind="ExternalInput"
    ).ap()
    slot_ap = nc.dram_tensor(
        "slotf", [NW, 128, CHUNKS], f32, kind="ExternalInput"
    ).ap()
    cnt_ap = nc.dram_tensor(
        "counts", [1, 32 * NPASS], i32, kind="ExternalInput"
    ).ap()
    iota_ap = nc.dram_tensor(
        "iota", [128, 128], f32, kind="ExternalInput"
    ).ap()
    out_ap = nc.dram_tensor("out", [BPC, D], f32, kind="ExternalOutput").ap()

    with tile.TileContext(nc) as tc:
        with (
            tc.tile_pool(name="iop", bufs=1) as iop,
            tc.tile_pool(name="inp", bufs=8) as inp,
            tc.tile_pool(name="gp", bufs=GBUFS) as gp,
            tc.tile_pool(name="ap_", bufs=5) as ap_,
            tc.tile_pool(name="op", bufs=2) as op,
            tc.tile_pool(name="pp", bufs=1, space="PSUM") as pp,
        ):
            # iota comes from HBM (host-filled) rather than nc.gpsimd.iota:
            # the Pool engine is the bottleneck and the iota kernel costs a
            # Q7 library swap at startup.
            iota_s = iop.tile([128, 128], f32, name="iota_s")
            nc.sync.dma_start(out=iota_s[:], in_=iota_ap[:, :])
            # iota lives in PSUM: the per-window A-build then has only ONE
            # SBUF input stream, so the DVE stays in 1-port mode and never
            # takes the SBUF port pair it shares with GpSimd. In 2-port mode
            # each 4.4us A-build fully blocked Q7 descriptor generation
            # (exclusive lock), stretching the window cadence.
            iota_p = pp.tile([128, 128], f32, name="iota_p", tag="iota_p")
            nc.vector.tensor_copy(iota_p[:], iota_s[:])

            ct = iop.tile([1, 32 * NPASS], i32, name="ct")
            nc.sync.dma_start(out=ct[:], in_=cnt_ap[:, :])
            # 8 count registers, loaded once per 2 windows: each Q7
            # TENSOR_LOAD round-trip occupies the Pool pipeline
            cregs = [
                nc.gpsimd.alloc_register(f"gather_cnt{q}") for q in range(8)
            ]

            psums = [
                pp.tile([128, D], f32, name=f"ps{g}", tag=f"ps{g}")
                for g in range(NPASS)
            ]

            for w in range(NW):
                lt = inp.tile([128, CALL_IDX // 16], i16, name="lt", tag="lt")
                nc.sync.dma_start(out=lt[:], in_=loc_ap[w])
                st = inp.tile([128, CHUNKS], f32, name="st", tag="st")
                nc.sync.dma_start(out=st[:], in_=slot_ap[w])

                A = ap_.tile([128, CHUNKS, 128], bf16, name="A", tag="A")
                iota_bc = iota_p[:].unsqueeze(1).broadcast_to([128, CHUNKS, 128])
                st_bc = st[:].unsqueeze(2).broadcast_to([128, CHUNKS, 128])
                nc.vector.tensor_tensor(A[:], iota_bc, st_bc, Alu.is_equal)

                if w % 2 == 0:
                    # one TENSOR_LOAD fills the count registers for 2 windows
                    nc.gpsimd.reg_load(
                        cregs, ct[0:1, w * NPASS : (w + 2) * NPASS]
                    )
                for grp in range(NPASS):
                    # one gather per (window, batch group), queue = grp so
                    # the 4 Q7 core pairs generate descriptors in parallel
                    g = gp.tile([128, SUBC, D], bf16, name="g", tag="g")
                    cwg = int(caps[w][grp])
                    nc.gpsimd.dma_gather(
                        g[:],
                        tb_ap[w * W : (w + 1) * W, :],
                        lt[:, grp * (CAP // 16) : grp * (CAP // 16) + cwg // 16],
                        cwg,
                        cregs[(w % 2) * NPASS + grp],
                        D,
                        queue_num=grp,
                    )
                    for c in range(SUBC):
                        nc.tensor.matmul(
                            psums[grp][:],
                            A[:, grp * SUBC + c, :],
                            g[:, c, :],
                            start=(w == 0 and c == 0),
                            stop=(w == NW - 1 and c == SUBC - 1),
                        )

            for grp in range(NPASS):
                outs = op.tile([128, D], f32, name="outs", tag="outs")
                nc.vector.tensor_copy(outs[:], psums[grp][:])
                nc.sync.dma_start(
                    out=out_ap[grp * 128 : (grp + 1) * 128, :], in_=outs[:]
                )

    nc.compile()
    return nc


def _run(x, table, trace=False):
    from concourse.bass_utils import run_bass_kernel_spmd

    x_np = np.asarray(x)
    # pad the table to NW*W rows so every gather window is a full 32768,
    # converted to bf16 (rel tolerance is 2e-2; bf16 error ~4e-3)
    import ml_dtypes

    tb = np.zeros((NW * W, D), dtype=np.uint16)
    tb[:V] = _f32_to_bf16(np.asarray(table, dtype=np.float32))
    tb16 = tb.view(ml_dtypes.bfloat16)
    iota = np.broadcast_to(
        np.arange(128, dtype=np.float32), (128, 128)
    ).copy()
    preps = [_host_prep(x_np[c * BPC : (c + 1) * BPC]) for c in range(NCORES)]
    # per-call static num_idxs: multiple of 16, >896 (so the output still
    # rounds to 8 chunks), covering the max count across the 8 cores
    maxc = np.max([p[2] for p in preps], axis=0).reshape(NW, NPASS)
    caps = np.clip((maxc + 15) // 16 * 16, 912, CAP).tolist()

    key = ("nc", str(caps))
    if key not in _cache:
        _cache[key] = _build(caps)
    nc = _cache[key]

    in_maps = []
    for c in range(NCORES):
        loc16, slotf, cnts = preps[c]
        cpad = np.full((1, 32 * NPASS), CAP, dtype=np.int32)
        cpad[0, : NW * NPASS] = cnts[0]
        in_maps.append(
            {
                "table": tb16,
                "loc16": loc16,
                "slotf": slotf,
                "counts": cpad,
                "iota": iota,
            }
        )
    res = run_bass_kernel_spmd(nc, in_maps, list(range(NCORES)), trace=trace)
    out = np.concatenate(
        [res.results[c]["out"] for c in range(NCORES)], axis=0
    ).astype(np.float32)
    return out, res


def kernel(x, table):
    out, _ = _run(x, table, trace=False)
    return out
